# revision 1
# baseline (speedup 1.0000x reference)
"""Multi-head attention block (B=4, N=2048, C=1024, H=16) on 8 trn2 cores.

Sharding: core c handles batch c//2 and heads (c%2)*8 .. (c%2)*8+8
(data parallel on B, tensor parallel on heads). Each core computes
qkv projections for its 8 heads, attention, and a partial output
projection (row-parallel over W_proj); the host sums the two partial
projections per batch and adds b_proj. The host also pre-transposes
x (ships xT) and pre-casts weights/activations to bf16 — pure data
layout/sharding prep.

Per-core dataflow (layouts chosen so no on-device transposes are
needed):
  qT/kT[hd, m] = Wqk.T @ x.T   (W-stationary, bf16, psum-accum over k)
  v[n, hd]     = x @ Wv        (xT-stationary, bf16)
  St[n, m]     = k @ q.T       (kT-stationary, bf16, 2-head row-packed)
  E = exp(St/8)                (ScalarE, fused scale, 1024-wide PSUM
                                reads across both heads' banks, bf16 out)
  av[d, m]     = v.T @ E       (bf16, 2-head col-packed, psum-accum over n)
  sums[m]      = ones64.T @ E  (replicated across 64 partitions by the
                                PE so no partition-broadcast is needed)
  att[d, m]    = av * approx_recip(sums)   (DVE)
  out_part     = att.T @ Wp    (bf16, psum-accum over head pairs)
"""

import numpy as np
import ml_dtypes

import concourse.bass as bass
import concourse.mybir as mybir
import concourse.tile as tile
from concourse import bacc
from concourse.bass_utils import run_bass_kernel_spmd

F32 = mybir.dt.float32
BF16 = mybir.dt.bfloat16
EXP = mybir.ActivationFunctionType.Exp

N = 2048          # sequence length
C = 1024          # model dim
DH = 64           # head dim
HPC = 8           # heads per core
P = 128           # partitions
NT = N // P       # 16 n/m tiles
KT = C // P       # 8 contraction tiles for qkv
MC = N // 512     # 4 m-chunks of 512
PAIRS = HPC // 2  # 4 head pairs
SCALE = 1.0 / np.sqrt(DH)


def _emit(nc, tc, ctx):
    xT_d = nc.dram_tensor("xT", [C, N], BF16, kind="ExternalInput").ap()
    wqk_d = nc.dram_tensor("wqk", [C, 2 * HPC * DH], BF16, kind="ExternalInput").ap()
    wv_d = nc.dram_tensor("wv", [C, HPC * DH], BF16, kind="ExternalInput").ap()
    wp_d = nc.dram_tensor("wp", [HPC * DH, C], BF16, kind="ExternalInput").ap()
    out_d = nc.dram_tensor("out", [N, C], F32, kind="ExternalOutput").ap()

    # --- pools ---
    consts = ctx.enter_context(tc.tile_pool(name="consts", bufs=1))
    sb_xT = ctx.enter_context(tc.tile_pool(name="sb_xT", bufs=KT))
    sb_wqk = ctx.enter_context(tc.tile_pool(name="sb_wqk", bufs=9))
    sb_wv = ctx.enter_context(tc.tile_pool(name="sb_wv", bufs=KT))
    sb_wp = ctx.enter_context(tc.tile_pool(name="sb_wp", bufs=PAIRS))
    sb_v = ctx.enter_context(tc.tile_pool(name="sb_v", bufs=NT))
    sb_qkT = ctx.enter_context(tc.tile_pool(name="sb_qkT", bufs=4))
    sb_se = ctx.enter_context(tc.tile_pool(name="sb_se", bufs=6))
    sb_rc = ctx.enter_context(tc.tile_pool(name="sb_rc", bufs=2))
    sb_att = ctx.enter_context(tc.tile_pool(name="sb_att", bufs=PAIRS * MC))
    sb_out = ctx.enter_context(tc.tile_pool(name="sb_out", bufs=3))

    ps_sc = ctx.enter_context(tc.tile_pool(name="ps_sc", bufs=2, space="PSUM"))
    ps_av = ctx.enter_context(tc.tile_pool(name="ps_av", bufs=1, space="PSUM"))
    ps_sm = ctx.enter_context(tc.tile_pool(name="ps_sm", bufs=1, space="PSUM"))
    ps_small = ctx.enter_context(tc.tile_pool(name="ps_small", bufs=2, space="PSUM"))

    # --- constants ---
    ones_bf = consts.tile([P, DH], BF16)
    nc.vector.memset(ones_bf, 1.0)

    # --- resident inputs: xT (chunked so qkT can start on chunk 0) ---
    xT = [sb_xT.tile([P, N], BF16, tag="xT", name=f"xT{k}") for k in range(KT)]
    for mc in range(MC):
        for k in range(KT):
            nc.sync.dma_start(
                out=xT[k][:, mc * 512:(mc + 1) * 512],
                in_=xT_d[k * P:(k + 1) * P, mc * 512:(mc + 1) * 512],
            )
    wv_sb = []
    wp_sb = []

    def load_wv_wp():
        for k in range(KT):
            w = sb_wv.tile([P, HPC * DH], BF16, tag="wv", name=f"wv{k}")
            nc.sync.dma_start(out=w, in_=wv_d[k * P:(k + 1) * P, :])
            wv_sb.append(w)
        for p in range(PAIRS):
            wb = sb_wp.tile([P, C], BF16, tag="wp", name=f"wp{p}")
            nc.sync.dma_start(out=wb, in_=wp_d[p * P:(p + 1) * P, :])
            wp_sb.append(wb)

    # --- v production (one m-tile at a time) ---
    v_sb = [None] * NT

    def emit_v(m):
        ps = ps_small.tile([P, 512], F32, tag="ps_small", name=f"vps{m}")
        for k in range(KT):
            nc.tensor.matmul(
                ps, xT[k][:, m * P:(m + 1) * P], wv_sb[k],
                start=(k == 0), stop=(k == KT - 1),
            )
        vt = sb_v.tile([P, HPC * DH], BF16, tag="v", name=f"v{m}")
        nc.vector.tensor_copy(vt, ps)
        v_sb[m] = vt

    # --- qkT production for one pair ---
    def emit_qkT(p):
        qT = sb_qkT.tile([P, N], BF16, tag="qkT", name=f"qT{p}")
        kT = sb_qkT.tile([P, N], BF16, tag="qkT", name=f"kT{p}")
        for ct, dst in [(p, qT), (PAIRS + p, kT)]:
            wts = []
            for k in range(KT):
                w = sb_wqk.tile([P, P], BF16, tag="wqk", name=f"wqk{ct}_{k}")
                nc.sync.dma_start(
                    out=w, in_=wqk_d[k * P:(k + 1) * P, ct * P:(ct + 1) * P]
                )
                wts.append(w)
            for mc in range(MC):
                ps = ps_small.tile([P, 512], F32, tag="ps_small",
                                   name=f"qkps{ct}_{mc}")
                for k in range(KT):
                    nc.tensor.matmul(
                        ps, wts[k], xT[k][:, mc * 512:(mc + 1) * 512],
                        start=(k == 0), stop=(k == KT - 1),
                    )
                nc.vector.tensor_copy(dst[:, mc * 512:(mc + 1) * 512], ps)
        return qT, kT

    att_tiles = {}

    def emit_attention(p, qT, kT, mc, n_hook=None):
        av = ps_av.tile([P, 512], F32, tag="av", name=f"av{p}_{mc}")
        sm = ps_sm.tile([P, 512], F32, tag="sm", name=f"sm{p}_{mc}")
        LAG = 2  # av/sums trail scores/exp to hide the exp->av sem latency
        ses = {}

        def emit_avsm(n):
            se = ses.pop(n)
            first, last = (n == 0), (n == NT - 1)
            for h in range(2):
                hd = p * P + h * DH
                nc.tensor.matmul(
                    av[h * DH:(h + 1) * DH, :],
                    v_sb[n][:, hd:hd + DH],
                    se[:, h * 512:(h + 1) * 512],
                    start=first, stop=last, skip_group_check=True,
                )
            for h in range(2):
                nc.tensor.matmul(
                    sm[h * DH:(h + 1) * DH, :],
                    ones_bf,
                    se[:, h * 512:(h + 1) * 512],
                    start=first, stop=last, skip_group_check=True,
                )

        for n in range(NT):
            sc = ps_sc.tile([P, 1024], F32, tag="sc", name=f"sc{p}_{mc}_{n}")
            for h in range(2):
                lo, hi = h * DH, (h + 1) * DH
                nc.tensor.matmul(
                    sc[:, h * 512:(h + 1) * 512],
                    kT[lo:hi, n * P:(n + 1) * P],
                    qT[lo:hi, mc * 512:(mc + 1) * 512],
                    start=True, stop=True, skip_group_check=True,
                )
            se = sb_se.tile([P, 1024], BF16, tag="se", name=f"se{p}_{mc}_{n}")
            nc.scalar.activation(se, sc, EXP, scale=float(SCALE))
            ses[n] = se
            if n_hook is not None:
                n_hook(n)
            if n >= LAG:
                emit_avsm(n - LAG)
        for n in range(NT - LAG, NT):
            emit_avsm(n)
        # normalize: att = av * (1/sums); sums are PE-replicated across
        # all 64 partitions per head, so no partition broadcast is needed.
        rc = sb_rc.tile([P, 512], F32, tag="rc", name=f"rc{p}_{mc}")
        nc.vector.reciprocal_approx_fast(rc, sm)
        att = sb_att.tile([P, 512], BF16, tag="att", name=f"att{p}_{mc}")
        nc.vector.tensor_tensor(att, av, rc, op=mybir.AluOpType.mult)
        att_tiles[(p, mc)] = att

    def emit_proj(mc):
        for m4 in range(4):
            m = mc * 4 + m4
            ot = sb_out.tile([P, C], F32, tag="out", name=f"out{m}")
            for cc in range(2):
                ps = ps_small.tile([P, 512], F32, tag="ps_small",
                                   name=f"pps{m}_{cc}")
                for p in range(PAIRS):
                    nc.tensor.matmul(
                        ps,
                        att_tiles[(p, mc)][:, m4 * P:(m4 + 1) * P],
                        wp_sb[p][:, cc * 512:(cc + 1) * 512],
                        start=(p == 0), stop=(p == PAIRS - 1),
                    )
                nc.vector.tensor_copy(ot[:, cc * 512:(cc + 1) * 512], ps)
            nc.sync.dma_start(out=out_d[m * P:(m + 1) * P, :], in_=ot)

    # v for the first attention block is produced just-in-time inside its
    # n-loop (keeps ScalarE fed early); qkT for pair p+1 is produced during
    # pair p's attention; proj(mc) runs during pair 3's attention.
    qkT_cur = emit_qkT(0)
    load_wv_wp()

    def v_hook(n):
        if v_sb[n] is None:
            emit_v(n)

    for p in range(PAIRS):
        qkT_next = None
        for mc in range(MC):
            emit_attention(p, qkT_cur[0], qkT_cur[1], mc,
                           n_hook=v_hook if (p == 0 and mc == 0) else None)
            if mc == 1 and p + 1 < PAIRS:
                qkT_next = emit_qkT(p + 1)
            if p == PAIRS - 1:
                emit_proj(mc)
        if qkT_next is not None:
            qkT_cur = qkT_next


def build_nc():
    from contextlib import ExitStack

    nc = bacc.Bacc("TRN2", target_bir_lowering=False, debug=False, num_devices=8)
    with tile.TileContext(nc) as tc:
        with ExitStack() as ctx:
            _emit(nc, tc, ctx)
    nc.compile()
    return nc


_NC = None


def _in_maps(x, W_qkv, W_proj):
    bf = ml_dtypes.bfloat16
    in_maps = []
    for c in range(8):
        b, h0 = c // 2, (c % 2) * HPC * DH  # h0 = col offset (0 or 512)
        in_maps.append({
            "xT": np.ascontiguousarray(x[b].T).astype(bf),
            "wqk": np.ascontiguousarray(
                np.concatenate(
                    [W_qkv[:, h0:h0 + 512], W_qkv[:, C + h0:C + h0 + 512]],
                    axis=1,
                )
            ).astype(bf),
            "wv": np.ascontiguousarray(
                W_qkv[:, 2 * C + h0:2 * C + h0 + 512]
            ).astype(bf),
            "wp": np.ascontiguousarray(W_proj[h0:h0 + 512, :]).astype(bf),
        })
    return in_maps


def kernel(x, W_qkv, b_qkv, W_proj, b_proj):
    global _NC
    assert np.all(b_qkv == 0.0), "kernel assumes zero qkv bias"
    x = np.asarray(x, np.float32)
    W_qkv = np.asarray(W_qkv, np.float32)
    W_proj = np.asarray(W_proj, np.float32)
    b_proj = np.asarray(b_proj, np.float32)
    if _NC is None:
        _NC = build_nc()
    res = run_bass_kernel_spmd(_NC, _in_maps(x, W_qkv, W_proj), list(range(8)))
    out = np.empty((4, N, C), np.float32)
    for b in range(4):
        out[b] = res.results[2 * b]["out"] + res.results[2 * b + 1]["out"] + b_proj
    return out



# revision 4
# speedup vs baseline: 1.0642x; 1.0642x over previous
"""Multi-head attention block (B=4, N=2048, C=1024, H=16) on 8 trn2 cores.

Sharding: core c handles batch c//2 and heads (c%2)*8 .. (c%2)*8+8
(data parallel on B, tensor parallel on heads). Each core computes
qkv projections for its 8 heads, attention, and a partial output
projection (row-parallel over W_proj); the host sums the two partial
projections per batch and adds b_proj. The host also pre-transposes
x (ships xT) and pre-casts weights/activations to bf16 — pure data
layout/sharding prep.

Per-core dataflow (layouts chosen so no on-device transposes are
needed):
  qT/kT[hd, m] = Wqk.T @ x.T   (W-stationary, bf16, psum-accum over k)
  v[n, hd]     = x @ Wv        (xT-stationary, bf16)
  St[n, m]     = k @ q.T       (kT-stationary, bf16, 2-head row-tiled
                                concurrent pair on the PE array)
  E = exp(St/8)                (ScalarE, fused scale, 1024-wide PSUM
                                reads across both heads' banks, bf16 out)
  av[d, m]     = v.T @ E       (bf16, 2-head col-tiled concurrent pair,
                                psum-accum over n)
  sums[m]      = ones64.T @ E  (replicated across 64 partitions by the
                                PE so no partition-broadcast is needed)
  att[d, m]    = av * approx_recip(sums)   (DVE)
  out_part     = att.T @ Wp    (bf16, psum-accum over head pairs)

Scheduling (v2): the inner loop runs in 2-tile blocks (SC,SC -> exp,exp
-> AVSM,AVSM) to cut PE array mode switches; qkv-for-next-pair and
proj matmuls are spread evenly (~2 small work units per n-tile) via a
work queue so ScalarE (the exp bottleneck, ~1us per n-tile) never
starves; input DMAs are priority-ordered so the first score matmul can
start ~6us in; the exp activation table is preloaded at t=0.
"""

from collections import deque

import numpy as np
import ml_dtypes

import concourse.bass as bass
import concourse.mybir as mybir
import concourse.tile as tile
from concourse import bacc
from concourse.bass_utils import run_bass_kernel_spmd

F32 = mybir.dt.float32
BF16 = mybir.dt.bfloat16
EXP = mybir.ActivationFunctionType.Exp

N = 2048          # sequence length
C = 1024          # model dim
DH = 64           # head dim
HPC = 8           # heads per core
P = 128           # partitions
NT = N // P       # 16 n/m tiles
KT = C // P       # 8 contraction tiles for qkv
MC = N // 512     # 4 m-chunks of 512
PAIRS = HPC // 2  # 4 head pairs
SCALE = 1.0 / np.sqrt(DH)


def _emit(nc, tc, ctx):
    xT_d = nc.dram_tensor("xT", [C, N], BF16, kind="ExternalInput").ap()
    wqk_d = nc.dram_tensor("wqk", [C, 2 * HPC * DH], BF16, kind="ExternalInput").ap()
    wv_d = nc.dram_tensor("wv", [C, HPC * DH], BF16, kind="ExternalInput").ap()
    wp_d = nc.dram_tensor("wp", [HPC * DH, C], BF16, kind="ExternalInput").ap()
    out_d = nc.dram_tensor("out", [N, C], F32, kind="ExternalOutput").ap()

    # --- pools ---
    consts = ctx.enter_context(tc.tile_pool(name="consts", bufs=1))
    sb_xT = ctx.enter_context(tc.tile_pool(name="sb_xT", bufs=KT))
    sb_wqk = ctx.enter_context(tc.tile_pool(name="sb_wqk", bufs=17))
    sb_wv = ctx.enter_context(tc.tile_pool(name="sb_wv", bufs=KT))
    sb_wp = ctx.enter_context(tc.tile_pool(name="sb_wp", bufs=PAIRS))
    sb_v = ctx.enter_context(tc.tile_pool(name="sb_v", bufs=NT))
    sb_qkT = ctx.enter_context(tc.tile_pool(name="sb_qkT", bufs=4))
    sb_se = ctx.enter_context(tc.tile_pool(name="sb_se", bufs=6))
    sb_rc = ctx.enter_context(tc.tile_pool(name="sb_rc", bufs=2))
    sb_att = ctx.enter_context(tc.tile_pool(name="sb_att", bufs=PAIRS * MC))
    sb_out = ctx.enter_context(tc.tile_pool(name="sb_out", bufs=4))

    ps_sc = ctx.enter_context(tc.tile_pool(name="ps_sc", bufs=2, space="PSUM"))
    ps_av = ctx.enter_context(tc.tile_pool(name="ps_av", bufs=1, space="PSUM"))
    ps_sm = ctx.enter_context(tc.tile_pool(name="ps_sm", bufs=1, space="PSUM"))
    ps_small = ctx.enter_context(tc.tile_pool(name="ps_small", bufs=2, space="PSUM"))

    # --- constants + exp activation-table preload (hides the ~2.7us
    # ACT_TABLE_LOAD under the initial DMA wait) ---
    ones_bf = consts.tile([P, DH], BF16)
    nc.vector.memset(ones_bf, 1.0)
    warm_in = consts.tile([P, 8], F32)
    warm_out = consts.tile([P, 8], BF16)
    nc.vector.memset(warm_in, 0.0)
    nc.scalar.activation(warm_out, warm_in, EXP, scale=1.0)

    # --- input DMAs, priority-ordered: xT chunk 0, then pair-0 qk
    # weights, then wv (needed for JIT v in mc0), then the rest ---
    xT = [sb_xT.tile([P, N], BF16, tag="xT", name=f"xT{k}") for k in range(KT)]

    def load_xT_chunk(mc):
        for k in range(KT):
            nc.sync.dma_start(
                out=xT[k][:, mc * 512:(mc + 1) * 512],
                in_=xT_d[k * P:(k + 1) * P, mc * 512:(mc + 1) * 512],
            )

    wqk_loaded = {}

    def load_wqk(ct):
        wts = []
        for k in range(KT):
            w = sb_wqk.tile([P, P], BF16, tag="wqk", name=f"wqk{ct}_{k}")
            nc.sync.dma_start(
                out=w, in_=wqk_d[k * P:(k + 1) * P, ct * P:(ct + 1) * P]
            )
            wts.append(w)
        wqk_loaded[ct] = wts

    wv_sb = []
    wp_sb = []

    def load_wv():
        for k in range(KT):
            w = sb_wv.tile([P, HPC * DH], BF16, tag="wv", name=f"wv{k}")
            nc.sync.dma_start(out=w, in_=wv_d[k * P:(k + 1) * P, :])
            wv_sb.append(w)

    def load_wp():
        for p in range(PAIRS):
            wb = sb_wp.tile([P, C], BF16, tag="wp", name=f"wp{p}")
            nc.sync.dma_start(out=wb, in_=wp_d[p * P:(p + 1) * P, :])
            wp_sb.append(wb)

    load_xT_chunk(0)
    load_wqk(0)          # q weights, pair 0
    load_wqk(PAIRS)      # k weights, pair 0
    load_wv()
    for mc in range(1, MC):
        load_xT_chunk(mc)
    load_wp()

    # --- work queue of small PE bursts (1 matmul or 1 cast/dma each),
    # drained a couple per n-tile so qkv/proj never bunch up ---
    work_q = deque()

    def drain(k):
        for _ in range(k):
            if work_q:
                work_q.popleft()()

    # --- v production (one m-tile at a time, JIT during p0 mc0) ---
    v_sb = [None] * NT

    def emit_v(m):
        ps = ps_small.tile([P, 512], F32, tag="ps_small", name=f"vps{m}")
        for k in range(KT):
            nc.tensor.matmul(
                ps, xT[k][:, m * P:(m + 1) * P], wv_sb[k],
                start=(k == 0), stop=(k == KT - 1),
            )
        vt = sb_v.tile([P, HPC * DH], BF16, tag="v", name=f"v{m}")
        nc.vector.tensor_copy(vt, ps)
        v_sb[m] = vt

    # --- qkT production: pair 0 up front; pairs 1-3 spread via work_q ---
    def emit_qkT_chain(ct, dst, mc):
        wts = wqk_loaded[ct]
        ps = ps_small.tile([P, 512], F32, tag="ps_small", name=f"qkps{ct}_{mc}")
        for k in range(KT):
            nc.tensor.matmul(
                ps, wts[k], xT[k][:, mc * 512:(mc + 1) * 512],
                start=(k == 0), stop=(k == KT - 1),
            )
        nc.vector.tensor_copy(dst[:, mc * 512:(mc + 1) * 512], ps)

    def emit_qkT_now(p):
        qT = sb_qkT.tile([P, N], BF16, tag="qkT", name=f"qT{p}")
        kT = sb_qkT.tile([P, N], BF16, tag="qkT", name=f"kT{p}")
        for ct, dst in [(PAIRS + p, kT), (p, qT)]:
            for mc in range(MC):
                emit_qkT_chain(ct, dst, mc)
        return qT, kT

    def push_qkT(p):
        """Queue pair-p qkT production as spread work units."""
        load_wqk(p)
        load_wqk(PAIRS + p)
        qT = sb_qkT.tile([P, N], BF16, tag="qkT", name=f"qT{p}")
        kT = sb_qkT.tile([P, N], BF16, tag="qkT", name=f"kT{p}")

        def unit(ct, dst, mc, k):
            def run():
                # re-derive the psum tile per chain via a dict closure
                key = (ct, mc)
                if key not in chain_ps:
                    chain_ps[key] = ps_small.tile(
                        [P, 512], F32, tag="ps_small", name=f"qkps{ct}_{mc}"
                    )
                nc.tensor.matmul(
                    chain_ps[key], wqk_loaded[ct][k],
                    xT[k][:, mc * 512:(mc + 1) * 512],
                    start=(k == 0), stop=(k == KT - 1),
                )
                if k == KT - 1:
                    nc.vector.tensor_copy(
                        dst[:, mc * 512:(mc + 1) * 512], chain_ps.pop(key)
                    )
            return run

        chain_ps = {}
        for ct, dst in [(PAIRS + p, kT), (p, qT)]:
            for mc in range(MC):
                for k in range(KT):
                    work_q.append(unit(ct, dst, mc, k))
        return qT, kT

    # --- proj: spread via work_q too ---
    att_tiles = {}

    def push_proj(mc):
        for m4 in range(4):
            m = mc * 4 + m4
            ot = sb_out.tile([P, C], F32, tag="out", name=f"out{m}")

            def unit(m, m4, ot, cc, p):
                def run():
                    key = (m, cc)
                    if key not in proj_ps:
                        proj_ps[key] = ps_small.tile(
                            [P, 512], F32, tag="ps_small", name=f"pps{m}_{cc}"
                        )
                    nc.tensor.matmul(
                        proj_ps[key],
                        att_tiles[(p, mc)][:, m4 * P:(m4 + 1) * P],
                        wp_sb[p][:, cc * 512:(cc + 1) * 512],
                        start=(p == 0), stop=(p == PAIRS - 1),
                    )
                    if p == PAIRS - 1:
                        nc.vector.tensor_copy(
                            ot[:, cc * 512:(cc + 1) * 512], proj_ps.pop(key)
                        )
                return run

            def dma_unit(m, ot):
                def run():
                    nc.sync.dma_start(out=out_d[m * P:(m + 1) * P, :], in_=ot)
                return run

            for cc in range(2):
                for p in range(PAIRS):
                    work_q.append(unit(m, m4, ot, cc, p))
            work_q.append(dma_unit(m, ot))

    proj_ps = {}

    # --- attention for one (pair, mc): 2-tile blocks ---
    def emit_attention(p, qT, kT, mc, v_jit=False, budget=2):
        av = ps_av.tile([P, 512], F32, tag="av", name=f"av{p}_{mc}")
        sm = ps_sm.tile([P, 512], F32, tag="sm", name=f"sm{p}_{mc}")
        ses = {}

        def emit_sc_exp(n):
            sc = ps_sc.tile([P, 1024], F32, tag="sc", name=f"sc{p}_{mc}_{n}")
            for h in range(2):
                lo, hi = h * DH, (h + 1) * DH
                nc.tensor.matmul(
                    sc[:, h * 512:(h + 1) * 512],
                    kT[lo:hi, n * P:(n + 1) * P],
                    qT[lo:hi, mc * 512:(mc + 1) * 512],
                    start=True, stop=True, skip_group_check=True,
                )
            return sc

        def emit_exp(n, sc):
            se = sb_se.tile([P, 1024], BF16, tag="se", name=f"se{p}_{mc}_{n}")
            nc.scalar.activation(se, sc, EXP, scale=float(SCALE))
            ses[n] = se

        def emit_avsm(n):
            se = ses.pop(n)
            first, last = (n == 0), (n == NT - 1)
            for h in range(2):
                hd = p * P + h * DH
                nc.tensor.matmul(
                    av[h * DH:(h + 1) * DH, :],
                    v_sb[n][:, hd:hd + DH],
                    se[:, h * 512:(h + 1) * 512],
                    start=first, stop=last, skip_group_check=True,
                )
            for h in range(2):
                nc.tensor.matmul(
                    sm[h * DH:(h + 1) * DH, :],
                    ones_bf,
                    se[:, h * 512:(h + 1) * 512],
                    start=first, stop=last, skip_group_check=True,
                )

        for nb in range(0, NT, 2):
            if v_jit:
                # produce v two tiles ahead of the avsm consumer
                for n in (nb, nb + 1):
                    if v_sb[n] is None:
                        emit_v(n)
            scs = [emit_sc_exp(nb), emit_sc_exp(nb + 1)]
            emit_exp(nb, scs[0])
            emit_exp(nb + 1, scs[1])
            if nb >= 2:
                emit_avsm(nb - 2)
                emit_avsm(nb - 1)
            drain(budget)
        emit_avsm(NT - 2)
        emit_avsm(NT - 1)
        # normalize: att = av * (1/sums); sums are PE-replicated across
        # all 64 partitions per head, so no partition broadcast is needed.
        rc = sb_rc.tile([P, 512], F32, tag="rc", name=f"rc{p}_{mc}")
        nc.vector.reciprocal_approx_fast(rc, sm)
        att = sb_att.tile([P, 512], BF16, tag="att", name=f"att{p}_{mc}")
        nc.vector.tensor_tensor(att, av, rc, op=mybir.AluOpType.mult)
        att_tiles[(p, mc)] = att

    # --- main schedule ---
    # pair 0 qkT up front; qkT(p+1) spread across pair p's mc1-3;
    # proj(mc) spread across pair 3's mc+1 (proj(3) drains at the end).
    qkT_cur = emit_qkT_now(0)
    qkT_next = None

    for p in range(PAIRS):
        for mc in range(MC):
            if mc == 1 and p + 1 < PAIRS:
                qkT_next = push_qkT(p + 1)
            if p == PAIRS - 1 and mc >= 1:
                push_proj(mc - 1)
            emit_attention(
                p, qkT_cur[0], qkT_cur[1], mc,
                v_jit=(p == 0 and mc == 0),
                budget=(3 if p == PAIRS - 1 else 2),
            )
        if qkT_next is not None:
            qkT_cur, qkT_next = qkT_next, None
    push_proj(MC - 1)
    drain(len(work_q))


def build_nc():
    from contextlib import ExitStack

    nc = bacc.Bacc("TRN2", target_bir_lowering=False, debug=False, num_devices=8)
    with tile.TileContext(nc) as tc:
        with ExitStack() as ctx:
            _emit(nc, tc, ctx)
    nc.compile()
    return nc


_NC = None


def _in_maps(x, W_qkv, W_proj):
    bf = ml_dtypes.bfloat16
    in_maps = []
    for c in range(8):
        b, h0 = c // 2, (c % 2) * HPC * DH  # h0 = col offset (0 or 512)
        in_maps.append({
            "xT": np.ascontiguousarray(x[b].T).astype(bf),
            "wqk": np.ascontiguousarray(
                np.concatenate(
                    [W_qkv[:, h0:h0 + 512], W_qkv[:, C + h0:C + h0 + 512]],
                    axis=1,
                )
            ).astype(bf),
            "wv": np.ascontiguousarray(
                W_qkv[:, 2 * C + h0:2 * C + h0 + 512]
            ).astype(bf),
            "wp": np.ascontiguousarray(W_proj[h0:h0 + 512, :]).astype(bf),
        })
    return in_maps


def kernel(x, W_qkv, b_qkv, W_proj, b_proj):
    global _NC
    assert np.all(b_qkv == 0.0), "kernel assumes zero qkv bias"
    x = np.asarray(x, np.float32)
    W_qkv = np.asarray(W_qkv, np.float32)
    W_proj = np.asarray(W_proj, np.float32)
    b_proj = np.asarray(b_proj, np.float32)
    if _NC is None:
        _NC = build_nc()
    res = run_bass_kernel_spmd(_NC, _in_maps(x, W_qkv, W_proj), list(range(8)))
    out = np.empty((4, N, C), np.float32)
    for b in range(4):
        out[b] = res.results[2 * b]["out"] + res.results[2 * b + 1]["out"] + b_proj
    return out


# revision 8
# speedup vs baseline: 1.0741x; 1.0092x over previous
"""Multi-head attention block (B=4, N=2048, C=1024, H=16) on 8 trn2 cores.

Sharding: core c handles batch c//2 and heads (c%2)*8 .. (c%2)*8+8
(data parallel on B, tensor parallel on heads). Each core computes
qkv projections for its 8 heads, attention, and a partial output
projection (row-parallel over W_proj); the host sums the two partial
projections per batch and adds b_proj. The host also pre-transposes /
re-tiles x and the weights into DMA-friendly layouts (8KB per-partition
contiguous rows) and pre-casts to bf16 — pure data layout/sharding prep.

Per-core dataflow (layouts chosen so no on-device transposes are
needed):
  qT/kT[hd, m] = Wqk.T @ x.T   (W-stationary, bf16, psum-accum over k)
  v[n, hd]     = x @ Wv        (xT-stationary, bf16, split in two
                                pair-halves so only half is needed early)
  St[n, m]     = k @ q.T       (kT-stationary, bf16, 2-head row-tiled
                                concurrent pair on the PE array)
  E = exp(St/8)                (ScalarE, fused scale, 1024-wide PSUM
                                reads across both heads' banks, bf16 out)
  av[d, m]     = v.T @ E       (bf16, 2-head col-tiled concurrent pair,
                                psum-accum over n)
  sums[m]      = ones64.T @ E  (replicated across 64 partitions by the
                                PE so no partition-broadcast is needed)
  att[d, m]    = av * approx_recip(sums)   (DVE)
  out_part     = att.T @ Wp    (bf16, psum-accum over head pairs)

Scheduling (v3): each pair runs one continuous 64-tile loop (SC/exp at
tile t, av/sm at t-2, the per-mc normalize inline) so there is no
per-mc pipeline bubble; qkv-for-next-pair, the deferred v-half, and
proj matmuls are spread evenly (~2 small work units per n-tile) via a
work queue so ScalarE (the exp bottleneck, ~1us per n-tile) never
starves; input DMAs are priority-ordered and big-packet; the exp
activation table is preloaded at t=0.
"""

from collections import deque

import numpy as np
import ml_dtypes

import concourse.bass as bass
import concourse.mybir as mybir
import concourse.tile as tile
from concourse import bacc
from concourse.bass_utils import run_bass_kernel_spmd

F32 = mybir.dt.float32
BF16 = mybir.dt.bfloat16
EXP = mybir.ActivationFunctionType.Exp

N = 2048          # sequence length
C = 1024          # model dim
DH = 64           # head dim
HPC = 8           # heads per core
P = 128           # partitions
NT = N // P       # 16 n/m tiles
KT = C // P       # 8 contraction tiles for qkv
MC = N // 512     # 4 m-chunks of 512
PAIRS = HPC // 2  # 4 head pairs
SCALE = 1.0 / np.sqrt(DH)
LAG = 2           # tiles the av/sm consumer trails the sc/exp producer


def _emit(nc, tc, ctx):
    # host-retiled inputs (see _in_maps):
    #  xTc: [4*128, 4096]  row mc*128+p, col k*512+j  = x[mc*512+j, k*128+p]
    #  wqk: [128, 8192]    row p, col ct*1024+k*128+c = Wqk_cat[k*128+p, ct*128+c]
    #  wv:  [128, 4096]    row p, col k*512+c         = Wv[k*128+p, c]
    #  wp:  [512, 1024]    as-is
    xTc_d = nc.dram_tensor("xTc", [MC * P, KT * 512], BF16, kind="ExternalInput").ap()
    wqk_d = nc.dram_tensor("wqk", [P, 8 * 1024], BF16, kind="ExternalInput").ap()
    wv_d = nc.dram_tensor("wv", [P, KT * 512], BF16, kind="ExternalInput").ap()
    wp_d = nc.dram_tensor("wp", [HPC * DH, C], BF16, kind="ExternalInput").ap()
    out_d = nc.dram_tensor("out", [N, C], F32, kind="ExternalOutput").ap()

    # --- pools ---
    consts = ctx.enter_context(tc.tile_pool(name="consts", bufs=1))
    sb_xT = ctx.enter_context(tc.tile_pool(name="sb_xT", bufs=MC))
    sb_wqk = ctx.enter_context(tc.tile_pool(name="sb_wqk", bufs=5))
    sb_wv = ctx.enter_context(tc.tile_pool(name="sb_wv", bufs=1))
    sb_wp = ctx.enter_context(tc.tile_pool(name="sb_wp", bufs=PAIRS))
    sb_v = ctx.enter_context(tc.tile_pool(name="sb_v", bufs=2 * NT))
    sb_qkT = ctx.enter_context(tc.tile_pool(name="sb_qkT", bufs=4))
    sb_se = ctx.enter_context(tc.tile_pool(name="sb_se", bufs=6))
    sb_rc = ctx.enter_context(tc.tile_pool(name="sb_rc", bufs=2))
    sb_att = ctx.enter_context(tc.tile_pool(name="sb_att", bufs=PAIRS * MC))
    sb_out = ctx.enter_context(tc.tile_pool(name="sb_out", bufs=4))

    ps_sc = ctx.enter_context(tc.tile_pool(name="ps_sc", bufs=2, space="PSUM"))
    ps_av = ctx.enter_context(tc.tile_pool(name="ps_av", bufs=1, space="PSUM"))
    ps_sm = ctx.enter_context(tc.tile_pool(name="ps_sm", bufs=1, space="PSUM"))
    ps_small = ctx.enter_context(tc.tile_pool(name="ps_small", bufs=2, space="PSUM"))

    # --- constants + exp activation-table preload (hides the ~2.7us
    # ACT_TABLE_LOAD under the initial DMA wait) ---
    ones_bf = consts.tile([P, DH], BF16)
    nc.vector.memset(ones_bf, 1.0)
    warm_in = consts.tile([P, 8], F32)
    warm_out = consts.tile([P, 8], BF16)
    nc.vector.memset(warm_in, 0.0)
    nc.scalar.activation(warm_out, warm_in, EXP, scale=1.0)

    # --- input DMAs, priority-ordered ---
    xTc = [sb_xT.tile([P, KT * 512], BF16, tag="xT", name=f"xTc{mc}")
           for mc in range(MC)]
    wqk_loaded = {}

    def load_wqk(ct):
        w = sb_wqk.tile([P, 8 * P], BF16, tag="wqk", name=f"wqk{ct}")
        nc.sync.dma_start(out=w, in_=wqk_d[:, ct * 1024:(ct + 1) * 1024])
        wqk_loaded[ct] = w

    nc.sync.dma_start(out=xTc[0], in_=xTc_d[0:P, :])
    load_wqk(PAIRS)      # k weights, pair 0 (kT chains run first)
    load_wqk(0)          # q weights, pair 0
    wv_all = sb_wv.tile([P, KT * 512], BF16, tag="wv", name="wv")
    nc.sync.dma_start(out=wv_all, in_=wv_d)
    for mc in range(1, MC):
        nc.sync.dma_start(out=xTc[mc], in_=xTc_d[mc * P:(mc + 1) * P, :])
    wp_sb = []
    for p in range(PAIRS):
        wb = sb_wp.tile([P, C], BF16, tag="wp", name=f"wp{p}")
        nc.sync.dma_start(out=wb, in_=wp_d[p * P:(p + 1) * P, :])
        wp_sb.append(wb)

    def xT(k, col0, w):
        """AP over x.T[k*128:(k+1)*128, col0:col0+w] in the mc-chunked tile."""
        mc, j = divmod(col0, 512)
        assert j + w <= 512
        return xTc[mc][:, k * 512 + j:k * 512 + j + w]

    # --- work queue of small PE bursts, drained per n-tile ---
    work_q = deque()

    def drain(k):
        for _ in range(k):
            if work_q:
                work_q.popleft()()

    # --- v production, split in pair-halves (half=0 -> pairs 0/1) ---
    v_sb = [[None] * NT, [None] * NT]

    def emit_v(half, m):
        ps = ps_small.tile([P, 512], F32, tag="ps_small", name=f"vps{half}_{m}")
        for k in range(KT):
            nc.tensor.matmul(
                ps[:, 0:256], xT(k, m * P, P),
                wv_all[:, k * 512 + half * 256:k * 512 + half * 256 + 256],
                start=(k == 0), stop=(k == KT - 1),
            )
        vt = sb_v.tile([P, 256], BF16, tag="v", name=f"v{half}_{m}")
        nc.vector.tensor_copy(vt, ps[:, 0:256])
        v_sb[half][m] = vt

    def push_v(half):
        def unit(m, k):
            def run():
                key = m
                if key not in v_ps[half]:
                    v_ps[half][key] = ps_small.tile(
                        [P, 512], F32, tag="ps_small", name=f"vps{half}_{m}"
                    )
                ps = v_ps[half][key]
                nc.tensor.matmul(
                    ps[:, 0:256], xT(k, m * P, P),
                    wv_all[:, k * 512 + half * 256:k * 512 + half * 256 + 256],
                    start=(k == 0), stop=(k == KT - 1),
                )
                if k == KT - 1:
                    vt = sb_v.tile([P, 256], BF16, tag="v", name=f"v{half}_{m}")
                    nc.vector.tensor_copy(vt, ps[:, 0:256])
                    v_sb[half][m] = vt
                    del v_ps[half][key]
            return run
        for m in range(NT):
            for k in range(KT):
                work_q.append(unit(m, k))

    v_ps = [{}, {}]

    def v_ap(p, n, h):
        half, off = divmod(p * P + h * DH, 256)
        return v_sb[half][n][:, off:off + DH]

    # --- qkT production: pair 0 up front; pairs 1-3 spread via work_q ---
    def wts(ct, k):
        return wqk_loaded[ct][:, k * P:(k + 1) * P]

    def emit_qkT_chain(ct, dst, mc):
        ps = ps_small.tile([P, 512], F32, tag="ps_small", name=f"qkps{ct}_{mc}")
        for k in range(KT):
            nc.tensor.matmul(
                ps, wts(ct, k), xT(k, mc * 512, 512),
                start=(k == 0), stop=(k == KT - 1),
            )
        nc.vector.tensor_copy(dst[:, mc * 512:(mc + 1) * 512], ps)

    def emit_qkT_now(p):
        qT = sb_qkT.tile([P, N], BF16, tag="qkT", name=f"qT{p}")
        kT = sb_qkT.tile([P, N], BF16, tag="qkT", name=f"kT{p}")
        for ct, dst in [(PAIRS + p, kT), (p, qT)]:
            for mc in range(MC):
                emit_qkT_chain(ct, dst, mc)
        return qT, kT

    def push_qkT(p):
        load_wqk(PAIRS + p)
        load_wqk(p)
        qT = sb_qkT.tile([P, N], BF16, tag="qkT", name=f"qT{p}")
        kT = sb_qkT.tile([P, N], BF16, tag="qkT", name=f"kT{p}")
        chain_ps = {}

        def unit(ct, dst, mc, k):
            def run():
                key = (ct, mc)
                if key not in chain_ps:
                    chain_ps[key] = ps_small.tile(
                        [P, 512], F32, tag="ps_small", name=f"qkps{ct}_{mc}"
                    )
                nc.tensor.matmul(
                    chain_ps[key], wts(ct, k), xT(k, mc * 512, 512),
                    start=(k == 0), stop=(k == KT - 1),
                )
                if k == KT - 1:
                    nc.vector.tensor_copy(
                        dst[:, mc * 512:(mc + 1) * 512], chain_ps.pop(key)
                    )
            return run

        for ct, dst in [(PAIRS + p, kT), (p, qT)]:
            for mc in range(MC):
                for k in range(KT):
                    work_q.append(unit(ct, dst, mc, k))
        return qT, kT

    # --- proj, spread via work_q ---
    att_tiles = {}
    proj_ps = {}

    def push_proj(mc):
        for m4 in range(4):
            m = mc * 4 + m4
            ot = sb_out.tile([P, C], F32, tag="out", name=f"out{m}")

            def unit(m, m4, ot, cc, p):
                def run():
                    key = (m, cc)
                    if key not in proj_ps:
                        proj_ps[key] = ps_small.tile(
                            [P, 512], F32, tag="ps_small", name=f"pps{m}_{cc}"
                        )
                    nc.tensor.matmul(
                        proj_ps[key],
                        att_tiles[(p, mc)][:, m4 * P:(m4 + 1) * P],
                        wp_sb[p][:, cc * 512:(cc + 1) * 512],
                        start=(p == 0), stop=(p == PAIRS - 1),
                    )
                    if p == PAIRS - 1:
                        nc.vector.tensor_copy(
                            ot[:, cc * 512:(cc + 1) * 512], proj_ps.pop(key)
                        )
                return run

            def dma_unit(m, ot):
                def run():
                    nc.sync.dma_start(out=out_d[m * P:(m + 1) * P, :], in_=ot)
                return run

            for cc in range(2):
                for p in range(PAIRS):
                    work_q.append(unit(m, m4, ot, cc, p))
            work_q.append(dma_unit(m, ot))

    # --- one pair: continuous 64-tile loop ---
    def emit_pair(p, qT, kT, budget, v_jit, after_att=None):
        ses = {}
        av = sm = None

        def emit_sc_exp(t):
            mc, n = divmod(t, NT)
            sc = ps_sc.tile([P, 1024], F32, tag="sc", name=f"sc{p}_{t}")
            for h in range(2):
                lo, hi = h * DH, (h + 1) * DH
                nc.tensor.matmul(
                    sc[:, h * 512:(h + 1) * 512],
                    kT[lo:hi, n * P:(n + 1) * P],
                    qT[lo:hi, mc * 512:(mc + 1) * 512],
                    start=True, stop=True, skip_group_check=True,
                )
            se = sb_se.tile([P, 1024], BF16, tag="se", name=f"se{p}_{t}")
            nc.scalar.activation(se, sc, EXP, scale=float(SCALE))
            ses[t] = se

        def emit_avsm(t):
            nonlocal av, sm
            mc, n = divmod(t, NT)
            if n == 0:
                av = ps_av.tile([P, 512], F32, tag="av", name=f"av{p}_{mc}")
                sm = ps_sm.tile([P, 512], F32, tag="sm", name=f"sm{p}_{mc}")
            se = ses.pop(t)
            first, last = (n == 0), (n == NT - 1)
            for h in range(2):
                nc.tensor.matmul(
                    av[h * DH:(h + 1) * DH, :],
                    v_ap(p, n, h),
                    se[:, h * 512:(h + 1) * 512],
                    start=first, stop=last, skip_group_check=True,
                )
            for h in range(2):
                nc.tensor.matmul(
                    sm[h * DH:(h + 1) * DH, :],
                    ones_bf,
                    se[:, h * 512:(h + 1) * 512],
                    start=first, stop=last, skip_group_check=True,
                )
            if last:
                rc = sb_rc.tile([P, 512], F32, tag="rc", name=f"rc{p}_{mc}")
                nc.vector.reciprocal_approx_fast(rc, sm)
                att = sb_att.tile([P, 512], BF16, tag="att", name=f"att{p}_{mc}")
                nc.vector.tensor_tensor(att, av, rc, op=mybir.AluOpType.mult)
                att_tiles[(p, mc)] = att
                if after_att is not None:
                    after_att(mc)

        for t in range(4 * NT + LAG):
            if t < 4 * NT:
                if v_jit and t < NT and v_sb[0][t] is None:
                    emit_v(0, t)
                emit_sc_exp(t)
            if t >= LAG:
                emit_avsm(t - LAG)
            # v-jit tiles already saturate the PE (and ps_small); defer
            # queued work there so chains don't interlock on psum bufs
            if not (v_jit and t < NT):
                drain(budget)

    # --- main schedule ---
    qkT_cur = emit_qkT_now(0)
    qkT_next = push_qkT(1)
    push_v(1)            # second v-half, consumed from pair 2 on
    emit_pair(0, *qkT_cur, budget=2, v_jit=True)

    qkT_cur, qkT_next = qkT_next, push_qkT(2)
    emit_pair(1, *qkT_cur, budget=3, v_jit=False)

    qkT_cur, qkT_next = qkT_next, push_qkT(3)
    emit_pair(2, *qkT_cur, budget=2, v_jit=False)

    qkT_cur = qkT_next
    emit_pair(3, *qkT_cur, budget=4, v_jit=False,
              after_att=lambda mc: push_proj(mc))
    drain(len(work_q))


def build_nc():
    from contextlib import ExitStack

    nc = bacc.Bacc("TRN2", target_bir_lowering=False, debug=False, num_devices=8)
    with tile.TileContext(nc) as tc:
        with ExitStack() as ctx:
            _emit(nc, tc, ctx)
    nc.compile()
    return nc


_NC = None


def _in_maps(x, W_qkv, W_proj):
    bf = ml_dtypes.bfloat16
    in_maps = []
    for c in range(8):
        b, h0 = c // 2, (c % 2) * HPC * DH  # h0 = col offset (0 or 512)
        xt = np.ascontiguousarray(x[b].T)                       # [C, N]
        xtc = (xt.reshape(KT, P, MC, 512).transpose(2, 1, 0, 3)
               .reshape(MC * P, KT * 512))
        wqk_cat = np.concatenate(
            [W_qkv[:, h0:h0 + 512], W_qkv[:, C + h0:C + h0 + 512]], axis=1
        )                                                       # [C, 1024]
        wqk3 = (wqk_cat.reshape(KT, P, 8, P).transpose(1, 2, 0, 3)
                .reshape(P, 8 * 1024))
        wv = W_qkv[:, 2 * C + h0:2 * C + h0 + 512]              # [C, 512]
        wv2 = wv.reshape(KT, P, 512).transpose(1, 0, 2).reshape(P, KT * 512)
        in_maps.append({
            "xTc": np.ascontiguousarray(xtc).astype(bf),
            "wqk": np.ascontiguousarray(wqk3).astype(bf),
            "wv": np.ascontiguousarray(wv2).astype(bf),
            "wp": np.ascontiguousarray(W_proj[h0:h0 + 512, :]).astype(bf),
        })
    return in_maps


def kernel(x, W_qkv, b_qkv, W_proj, b_proj):
    global _NC
    assert np.all(b_qkv == 0.0), "kernel assumes zero qkv bias"
    x = np.asarray(x, np.float32)
    W_qkv = np.asarray(W_qkv, np.float32)
    W_proj = np.asarray(W_proj, np.float32)
    b_proj = np.asarray(b_proj, np.float32)
    if _NC is None:
        _NC = build_nc()
    res = run_bass_kernel_spmd(_NC, _in_maps(x, W_qkv, W_proj), list(range(8)))
    out = np.empty((4, N, C), np.float32)
    for b in range(4):
        out[b] = res.results[2 * b]["out"] + res.results[2 * b + 1]["out"] + b_proj
    return out


# revision 10
# speedup vs baseline: 1.1115x; 1.0348x over previous
"""Multi-head attention block (B=4, N=2048, C=1024, H=16) on 8 trn2 cores.

Sharding: core c handles batch c//2 and heads (c%2)*8 .. (c%2)*8+8
(data parallel on B, tensor parallel on heads). Each core computes
qkv projections for its 8 heads, attention, and a partial output
projection (row-parallel over W_proj); the host sums the two partial
projections per batch and adds b_proj. The host also pre-transposes /
re-tiles x and the weights into DMA-friendly layouts (2-8KB per-partition
contiguous rows) and pre-casts to bf16 — pure data layout/sharding prep.

Per-core dataflow (layouts chosen so no on-device transposes are
needed):
  qT/kT[hd, m] = Wqk.T @ x.T   (W-stationary, bf16, psum-accum over k)
  v[n, hd]     = x @ Wv        (xT-stationary, bf16)
  St[n, m]     = k @ q.T       (kT-stationary, bf16, 2-head row-tiled
                                concurrent pair on the PE array)
  E = exp(St/8)                (ScalarE, fused scale, 1024-wide PSUM
                                reads across both heads' banks, bf16 out)
  av[d, m]     = v.T @ E       (bf16, 2-head col-tiled concurrent pair,
                                psum-accum over n)
  sums[m]      = ones64.T @ E  (replicated across 64 partitions by the
                                PE so no partition-broadcast is needed)
  att[d, m]    = av * approx_recip(sums)   (DVE)
  out_part     = att.T @ Wp    (bf16, psum-accum over head pairs)

Scheduling (v4): each pair runs one continuous 64-tile loop in 2-tile
blocks ordered [SC,SC | exp,exp | AV,AV,AV,AV,SM,SM,SM,SM | qp burst]
to minimize PE array-tiling mode switches (row->col->full cycles cost
~100ns each); qkv-for-next-pair and proj matmuls are spread via a work
queue; input DMAs are priority-ordered, big-packet, and partition-split
for queue parallelism; the exp table is preloaded at t=0.
"""

from collections import deque

import numpy as np
import ml_dtypes

import concourse.bass as bass
import concourse.mybir as mybir
import concourse.tile as tile
from concourse import bacc
from concourse.bass_utils import run_bass_kernel_spmd

F32 = mybir.dt.float32
BF16 = mybir.dt.bfloat16
EXP = mybir.ActivationFunctionType.Exp

N = 2048          # sequence length
C = 1024          # model dim
DH = 64           # head dim
HPC = 8           # heads per core
P = 128           # partitions
NT = N // P       # 16 n/m tiles
KT = C // P       # 8 contraction tiles for qkv
MC = N // 512     # 4 m-chunks of 512
PAIRS = HPC // 2  # 4 head pairs
SCALE = 1.0 / np.sqrt(DH)
LAG = 2           # tiles the av/sm consumer trails the sc/exp producer


def _emit(nc, tc, ctx):
    # host-retiled inputs (see _in_maps):
    #  xTc: [4*128, 4096]  row mc*128+p, col k*512+j  = x[mc*512+j, k*128+p]
    #  wqk: [128, 8192]    row p, col ct*1024+k*128+c = Wqk_cat[k*128+p, ct*128+c]
    #  wv:  [128, 4096]    row p, col k*512+c         = Wv[k*128+p, c]
    #  wp:  [512, 1024]    as-is
    xTc_d = nc.dram_tensor("xTc", [MC * P, KT * 512], BF16, kind="ExternalInput").ap()
    wqk_d = nc.dram_tensor("wqk", [P, 8 * 1024], BF16, kind="ExternalInput").ap()
    wv_d = nc.dram_tensor("wv", [P, KT * 512], BF16, kind="ExternalInput").ap()
    wp_d = nc.dram_tensor("wp", [HPC * DH, C], BF16, kind="ExternalInput").ap()
    out_d = nc.dram_tensor("out", [N, C], F32, kind="ExternalOutput").ap()

    # --- pools ---
    consts = ctx.enter_context(tc.tile_pool(name="consts", bufs=1))
    sb_xT = ctx.enter_context(tc.tile_pool(name="sb_xT", bufs=MC))
    sb_wqk = ctx.enter_context(tc.tile_pool(name="sb_wqk", bufs=5))
    sb_wv = ctx.enter_context(tc.tile_pool(name="sb_wv", bufs=1))
    sb_wp = ctx.enter_context(tc.tile_pool(name="sb_wp", bufs=PAIRS))
    sb_v = ctx.enter_context(tc.tile_pool(name="sb_v", bufs=NT))
    sb_qkT = ctx.enter_context(tc.tile_pool(name="sb_qkT", bufs=4))
    sb_se = ctx.enter_context(tc.tile_pool(name="sb_se", bufs=6))
    sb_rc = ctx.enter_context(tc.tile_pool(name="sb_rc", bufs=2))
    sb_att = ctx.enter_context(tc.tile_pool(name="sb_att", bufs=PAIRS * MC))
    sb_out = ctx.enter_context(tc.tile_pool(name="sb_out", bufs=4))

    ps_sc = ctx.enter_context(tc.tile_pool(name="ps_sc", bufs=2, space="PSUM"))
    ps_av = ctx.enter_context(tc.tile_pool(name="ps_av", bufs=1, space="PSUM"))
    ps_sm = ctx.enter_context(tc.tile_pool(name="ps_sm", bufs=1, space="PSUM"))
    ps_small = ctx.enter_context(tc.tile_pool(name="ps_small", bufs=2, space="PSUM"))

    # --- constants + exp activation-table preload (hides the ~2.7us
    # ACT_TABLE_LOAD under the initial DMA wait) ---
    ones_bf = consts.tile([P, DH], BF16)
    nc.vector.memset(ones_bf, 1.0)
    warm_in = consts.tile([P, 8], F32)
    warm_out = consts.tile([P, 8], BF16)
    nc.vector.memset(warm_in, 0.0)
    nc.scalar.activation(warm_out, warm_in, EXP, scale=1.0)

    # --- input DMAs, priority-ordered; the critical first tiles are
    # partition-split so several DMA queues move them in parallel ---
    xTc = [sb_xT.tile([P, KT * 512], BF16, tag="xT", name=f"xTc{mc}")
           for mc in range(MC)]
    wqk_loaded = {}

    def load_wqk(ct, split=1):
        w = sb_wqk.tile([P, 8 * P], BF16, tag="wqk", name=f"wqk{ct}")
        step = P // split
        for i in range(split):
            sl = slice(i * step, (i + 1) * step)
            nc.sync.dma_start(out=w[sl, :], in_=wqk_d[sl, ct * 1024:(ct + 1) * 1024])
        wqk_loaded[ct] = w

    for i in range(4):
        sl = slice(i * 32, (i + 1) * 32)
        nc.sync.dma_start(out=xTc[0][sl, :], in_=xTc_d[sl, :])
    load_wqk(PAIRS, split=2)   # k weights, pair 0 (kT chains run first)
    load_wqk(0, split=2)       # q weights, pair 0
    wv_all = sb_wv.tile([P, KT * 512], BF16, tag="wv", name="wv")
    nc.sync.dma_start(out=wv_all, in_=wv_d)
    for mc in range(1, MC):
        nc.sync.dma_start(out=xTc[mc], in_=xTc_d[mc * P:(mc + 1) * P, :])
    wp_sb = []
    for p in range(PAIRS):
        wb = sb_wp.tile([P, C], BF16, tag="wp", name=f"wp{p}")
        nc.sync.dma_start(out=wb, in_=wp_d[p * P:(p + 1) * P, :])
        wp_sb.append(wb)

    def xT(k, col0, w):
        """AP over x.T[k*128:(k+1)*128, col0:col0+w] in the mc-chunked tile."""
        mc, j = divmod(col0, 512)
        assert j + w <= 512
        return xTc[mc][:, k * 512 + j:k * 512 + j + w]

    # --- work queue of small PE bursts, drained per block ---
    work_q = deque()

    def drain(k):
        for _ in range(k):
            if work_q:
                work_q.popleft()()

    # --- v production (JIT during p0 mc0) ---
    v_sb = [None] * NT

    def emit_v(m):
        ps = ps_small.tile([P, 512], F32, tag="ps_small", name=f"vps{m}")
        for k in range(KT):
            nc.tensor.matmul(
                ps, xT(k, m * P, P), wv_all[:, k * 512:(k + 1) * 512],
                start=(k == 0), stop=(k == KT - 1),
            )
        vt = sb_v.tile([P, HPC * DH], BF16, tag="v", name=f"v{m}")
        nc.vector.tensor_copy(vt, ps)
        v_sb[m] = vt

    # --- qkT production: pair 0 up front; pairs 1-3 spread via work_q ---
    def wts(ct, k):
        return wqk_loaded[ct][:, k * P:(k + 1) * P]

    def emit_qkT_chain(ct, dst, mc):
        ps = ps_small.tile([P, 512], F32, tag="ps_small", name=f"qkps{ct}_{mc}")
        for k in range(KT):
            nc.tensor.matmul(
                ps, wts(ct, k), xT(k, mc * 512, 512),
                start=(k == 0), stop=(k == KT - 1),
            )
        nc.vector.tensor_copy(dst[:, mc * 512:(mc + 1) * 512], ps)

    def emit_qkT_now(p):
        qT = sb_qkT.tile([P, N], BF16, tag="qkT", name=f"qT{p}")
        kT = sb_qkT.tile([P, N], BF16, tag="qkT", name=f"kT{p}")
        for ct, dst in [(PAIRS + p, kT), (p, qT)]:
            for mc in range(MC):
                emit_qkT_chain(ct, dst, mc)
        return qT, kT

    def push_qkT(p):
        load_wqk(PAIRS + p)
        load_wqk(p)
        qT = sb_qkT.tile([P, N], BF16, tag="qkT", name=f"qT{p}")
        kT = sb_qkT.tile([P, N], BF16, tag="qkT", name=f"kT{p}")
        chain_ps = {}

        def unit(ct, dst, mc, k):
            def run():
                key = (ct, mc)
                if key not in chain_ps:
                    chain_ps[key] = ps_small.tile(
                        [P, 512], F32, tag="ps_small", name=f"qkps{ct}_{mc}"
                    )
                nc.tensor.matmul(
                    chain_ps[key], wts(ct, k), xT(k, mc * 512, 512),
                    start=(k == 0), stop=(k == KT - 1),
                )
                if k == KT - 1:
                    nc.vector.tensor_copy(
                        dst[:, mc * 512:(mc + 1) * 512], chain_ps.pop(key)
                    )
            return run

        for ct, dst in [(PAIRS + p, kT), (p, qT)]:
            for mc in range(MC):
                for k in range(KT):
                    work_q.append(unit(ct, dst, mc, k))
        return qT, kT

    # --- proj, spread via work_q ---
    att_tiles = {}
    proj_ps = {}

    def push_proj(mc):
        for m4 in range(4):
            m = mc * 4 + m4
            ot = sb_out.tile([P, C], F32, tag="out", name=f"out{m}")

            def unit(m, m4, ot, cc, p):
                def run():
                    key = (m, cc)
                    if key not in proj_ps:
                        proj_ps[key] = ps_small.tile(
                            [P, 512], F32, tag="ps_small", name=f"pps{m}_{cc}"
                        )
                    nc.tensor.matmul(
                        proj_ps[key],
                        att_tiles[(p, mc)][:, m4 * P:(m4 + 1) * P],
                        wp_sb[p][:, cc * 512:(cc + 1) * 512],
                        start=(p == 0), stop=(p == PAIRS - 1),
                    )
                    if p == PAIRS - 1:
                        nc.vector.tensor_copy(
                            ot[:, cc * 512:(cc + 1) * 512], proj_ps.pop(key)
                        )
                return run

            def dma_unit(m, ot):
                def run():
                    nc.sync.dma_start(out=out_d[m * P:(m + 1) * P, :], in_=ot)
                return run

            for cc in range(2):
                for p in range(PAIRS):
                    work_q.append(unit(m, m4, ot, cc, p))
            work_q.append(dma_unit(m, ot))

    # --- one pair: continuous 64-tile loop in 2-tile blocks ---
    def emit_pair(p, qT, kT, budget, v_jit, after_att=None):
        ses = {}
        av = sm = None

        def emit_sc(t):
            mc, n = divmod(t, NT)
            sc = ps_sc.tile([P, 1024], F32, tag="sc", name=f"sc{p}_{t}")
            for h in range(2):
                lo, hi = h * DH, (h + 1) * DH
                nc.tensor.matmul(
                    sc[:, h * 512:(h + 1) * 512],
                    kT[lo:hi, n * P:(n + 1) * P],
                    qT[lo:hi, mc * 512:(mc + 1) * 512],
                    start=True, stop=True, skip_group_check=True,
                )
            return sc

        def emit_exp(t, sc):
            se = sb_se.tile([P, 1024], BF16, tag="se", name=f"se{p}_{t}")
            nc.scalar.activation(se, sc, EXP, scale=float(SCALE))
            ses[t] = se

        def emit_av(t):
            nonlocal av
            mc, n = divmod(t, NT)
            if n == 0:
                av = ps_av.tile([P, 512], F32, tag="av", name=f"av{p}_{mc}")
            se = ses[t]
            first, last = (n == 0), (n == NT - 1)
            for h in range(2):
                nc.tensor.matmul(
                    av[h * DH:(h + 1) * DH, :],
                    v_sb[n][:, p * P + h * DH:p * P + (h + 1) * DH],
                    se[:, h * 512:(h + 1) * 512],
                    start=first, stop=last, skip_group_check=True,
                )

        def emit_sm(t):
            nonlocal sm
            mc, n = divmod(t, NT)
            if n == 0:
                sm = ps_sm.tile([P, 512], F32, tag="sm", name=f"sm{p}_{mc}")
            se = ses.pop(t)
            first, last = (n == 0), (n == NT - 1)
            for h in range(2):
                nc.tensor.matmul(
                    sm[h * DH:(h + 1) * DH, :],
                    ones_bf,
                    se[:, h * 512:(h + 1) * 512],
                    start=first, stop=last, skip_group_check=True,
                )
            if last:
                rc = sb_rc.tile([P, 512], F32, tag="rc", name=f"rc{p}_{mc}")
                nc.vector.reciprocal_approx_fast(rc, sm)
                att = sb_att.tile([P, 512], BF16, tag="att", name=f"att{p}_{mc}")
                nc.vector.tensor_tensor(att, av, rc, op=mybir.AluOpType.mult)
                att_tiles[(p, mc)] = att
                if after_att is not None:
                    after_att(mc)

        n_blocks = 4 * NT // 2 + 1   # 33 blocks: 32 produce + lag tail
        for b in range(n_blocks):
            t0, t1 = 2 * b, 2 * b + 1
            if t0 < 4 * NT:
                if v_jit and t0 < NT:
                    emit_v(t0)
                    emit_v(t1)
                scs = (emit_sc(t0), emit_sc(t1))
                emit_exp(t0, scs[0])
                emit_exp(t1, scs[1])
            if b >= 1:
                s0, s1 = t0 - LAG, t1 - LAG
                emit_av(s0)
                emit_av(s1)
                emit_sm(s0)
                emit_sm(s1)
            if not (v_jit and t0 < NT):
                drain(2 * budget)

    # --- main schedule ---
    qkT_cur = emit_qkT_now(0)
    qkT_next = push_qkT(1)
    emit_pair(0, *qkT_cur, budget=2, v_jit=True)

    qkT_cur, qkT_next = qkT_next, push_qkT(2)
    emit_pair(1, *qkT_cur, budget=2, v_jit=False)

    qkT_cur, qkT_next = qkT_next, push_qkT(3)
    emit_pair(2, *qkT_cur, budget=2, v_jit=False)

    qkT_cur = qkT_next
    emit_pair(3, *qkT_cur, budget=4, v_jit=False,
              after_att=lambda mc: push_proj(mc))
    drain(len(work_q))


def build_nc():
    from contextlib import ExitStack

    nc = bacc.Bacc("TRN2", target_bir_lowering=False, debug=False, num_devices=8)
    with tile.TileContext(nc) as tc:
        with ExitStack() as ctx:
            _emit(nc, tc, ctx)
    nc.compile()
    return nc


_NC = None


def _in_maps(x, W_qkv, W_proj):
    bf = ml_dtypes.bfloat16
    in_maps = []
    for c in range(8):
        b, h0 = c // 2, (c % 2) * HPC * DH  # h0 = col offset (0 or 512)
        xt = np.ascontiguousarray(x[b].T)                       # [C, N]
        xtc = (xt.reshape(KT, P, MC, 512).transpose(2, 1, 0, 3)
               .reshape(MC * P, KT * 512))
        wqk_cat = np.concatenate(
            [W_qkv[:, h0:h0 + 512], W_qkv[:, C + h0:C + h0 + 512]], axis=1
        )                                                       # [C, 1024]
        wqk3 = (wqk_cat.reshape(KT, P, 8, P).transpose(1, 2, 0, 3)
                .reshape(P, 8 * 1024))
        wv = W_qkv[:, 2 * C + h0:2 * C + h0 + 512]              # [C, 512]
        wv2 = wv.reshape(KT, P, 512).transpose(1, 0, 2).reshape(P, KT * 512)
        in_maps.append({
            "xTc": np.ascontiguousarray(xtc).astype(bf),
            "wqk": np.ascontiguousarray(wqk3).astype(bf),
            "wv": np.ascontiguousarray(wv2).astype(bf),
            "wp": np.ascontiguousarray(W_proj[h0:h0 + 512, :]).astype(bf),
        })
    return in_maps


def kernel(x, W_qkv, b_qkv, W_proj, b_proj):
    global _NC
    assert np.all(b_qkv == 0.0), "kernel assumes zero qkv bias"
    x = np.asarray(x, np.float32)
    W_qkv = np.asarray(W_qkv, np.float32)
    W_proj = np.asarray(W_proj, np.float32)
    b_proj = np.asarray(b_proj, np.float32)
    if _NC is None:
        _NC = build_nc()
    res = run_bass_kernel_spmd(_NC, _in_maps(x, W_qkv, W_proj), list(range(8)))
    out = np.empty((4, N, C), np.float32)
    for b in range(4):
        out[b] = res.results[2 * b]["out"] + res.results[2 * b + 1]["out"] + b_proj
    return out
